# revision 9
# baseline (speedup 1.0000x reference)
"""Kalman filter kernel for Trainium2 (8 NeuronCores, SPMD).

Key structure of the problem (B=512, T=2048, D=64):

The reference's scan carries ONLY the covariance P (batch-independent);
per-batch states never feed back into the recursion.  Each timestep's
output is therefore an affine function of that timestep's inputs:

    out[:, t] = x[:, t] @ F_t + eps[t-1] @ E_t + c_t

with 64x64 coefficient matrices (F_t, E_t) and bias c_t derived from the
Kalman gain K_t, which this module computes on host mirroring the
reference's float32 op order.

With Q = R = I (the actual inputs), the recursion is degenerate: K_1 = I
exactly, P becomes exactly 0 after step 1, inv(0) -> NaN, and every
reference output for t >= 2 is entirely NaN.  Timesteps whose
coefficients are non-finite provably produce all-NaN rows (any inner
product with an all-NaN matrix is NaN), so the kernel computes the
finite prefix on device and fills the NaN tail directly.

Device work per core (batch-sharded, 64 rows/core): for each finite t,
one K=128 matmul ([F_t;E_t].T @ [x.T;eps.T]) into PSUM, then a ScalarE
activation copy with per-partition bias c_t.  All per-core inputs are
packed into a single DMA-able block.
"""

import os

import numpy as np

import concourse.bass as bass
import concourse.tile as tile
from concourse import mybir
from concourse.bass_utils import run_bass_kernel_spmd
from concourse.vector_clock import ScopedClock

N_CORES = 8
D = 64


def _split_wait_drain_and_barrier(self, tick_clock, wait_clock):
    """Replacement for TileContext._drain_and_barrier.

    The stock version attaches one sync wait per live semaphore to the
    single kernel-tail Drain; this walrus build rejects instructions
    with more than one sync wait ("Too many sync wait commands").  Split
    the waits across single-wait SP nops so the drain itself needs none.
    """
    nc = self.nc
    probe = nc.sync.nop(nofuse=True, hint="tail_drain_waits")
    wait_clock.add_sem_waits(
        probe.ins, ScopedClock({None: tick_clock.global_clock})
    )
    si = probe.ins.sync_info
    if si is not None and len(si.on_wait) > 1:
        waits = list(si.on_wait)
        probe.ins.sync_info = mybir.SyncInfo(
            on_wait=[waits[0]], on_update=list(si.on_update)
        )
        for w in waits[1:]:
            extra = nc.sync.nop(nofuse=True, hint="tail_drain_waits")
            extra.ins.sync_info = mybir.SyncInfo(on_wait=[w], on_update=[])
    nc.sync.drain()
    nc.all_engine_barrier()
    assert self.sems is not None
    popped = nc._tile_sem_poison_stack.pop()
    assert popped is self._sem_poison
    nc.clear_and_free_semaphores(list(self.sems.allocated().values()))
    nc.all_engine_barrier()


tile.TileContext._drain_and_barrier = _split_wait_drain_and_barrier

# BassKernelResults of the most recent kernel() call (exec_time_ns is
# populated when KALMAN_TRACE=1).
LAST_RESULT = None


def _coeffs(W_t, b_t, Q, W_o, b_o, R, T):
    """Per-timestep linear coefficients mirroring the reference's float32
    op order.  Returns (T_active, Fs, Es, cs); reference outputs for
    t >= T_active are entirely non-finite."""
    f32 = np.float32
    I = np.eye(D, dtype=f32)
    nan_mat = np.full((D, D), np.nan, f32)
    try:
        L = np.linalg.cholesky(Q)
    except np.linalg.LinAlgError:
        L = nan_mat
    Fs, Es, cs = [], [], []

    # t = 0: out_0 = (x_0 @ W_t.T + b_t) @ W_o.T + b_o
    F0 = W_t.T @ W_o.T
    c0 = b_t @ W_o.T + b_o
    if not (np.isfinite(F0).all() and np.isfinite(c0).all()):
        return 0, Fs, Es, cs
    Fs.append(F0)
    Es.append(np.zeros((D, D), f32))
    cs.append(c0)

    P = I.copy()
    for t in range(1, T):
        P_pred = (Q @ P) @ Q.T
        S = (R @ P_pred) @ R.T
        try:
            S_inv = np.linalg.inv(S)
        except np.linalg.LinAlgError:
            S_inv = nan_mat
        K = (P_pred @ R.T) @ S_inv
        Z = I - W_o.T @ K.T
        F = (W_t.T @ Z + K.T) @ W_o.T
        E = L.T @ (Z @ W_o.T)
        c = (b_t @ Z - b_o @ K.T) @ W_o.T + b_o
        if not (np.isfinite(F).all() and np.isfinite(E).all() and np.isfinite(c).all()):
            return t, Fs, Es, cs
        Fs.append(F)
        Es.append(E)
        cs.append(c)
        P = P_pred - (K @ R) @ P_pred
        if not np.isfinite(P).all():
            # K (hence every output) is non-finite from t+1 on
            return t + 1, Fs, Es, cs
    return T, Fs, Es, cs


CW = 4 * D  # per-timestep column block in the packed input


def _build_nc(T_act, b_core, t_chunk):
    """One SPMD program: per t, psum[d, b] = lhsT.T @ rhs accumulated
    with the rank-1 bias c_t (x) ones, then DMA PSUM -> DRAM.

    pk layout (per core): [128, T_act*CW] where each timestep's CW=256
    column block is [lhsT (64 cols: F_t rows 0:64, E_t rows 64:128) |
    rhs (64 cols: x.T rows 0:64, eps.T rows 64:128) | c_t (row 0,
    64 cols) | ones (row 0, b_core cols)].  The bias lives in the
    matmul so no engine needs a second sync wait (the HW Activation
    encoding only fits one).
    """
    f32 = mybir.dt.float32
    nc = bass.Bass()
    pk = nc.declare_dram_parameter("pk", [128, T_act * CW], f32, isOutput=False)
    o = nc.declare_dram_parameter("o", [T_act, D, b_core], f32, isOutput=True)

    with tile.TileContext(nc) as tc:
        with (
            tc.tile_pool(name="io", bufs=2) as iop,
            tc.tile_pool(name="ops", bufs=4) as osp,
            tc.tile_pool(name="ps", bufs=8, space="PSUM") as psp,
        ):
            for t0 in range(0, T_act, t_chunk):
                tn = min(t_chunk, T_act - t0)
                pk_t = iop.tile([128, tn * CW], f32, tag="pk")
                nc.gpsimd.dma_start(
                    out=pk_t[:], in_=pk[:, t0 * CW : (t0 + tn) * CW]
                )
                ob = osp.tile([D, tn * b_core], f32, tag="o")
                for j in range(tn):
                    base = j * CW
                    ps_t = psp.tile([D, b_core], f32, tag="ps")
                    nc.tensor.matmul(
                        ps_t[:],
                        pk_t[:, base : base + D],
                        pk_t[:, base + D : base + 2 * D],
                        start=True,
                        stop=False,
                    )
                    nc.tensor.matmul(
                        ps_t[:],
                        pk_t[0:1, base + 2 * D : base + 3 * D],
                        pk_t[0:1, base + 3 * D : base + 3 * D + b_core],
                        start=False,
                        stop=True,
                    )
                    nc.vector.tensor_copy(
                        ob[:, j * b_core : (j + 1) * b_core], ps_t[:]
                    )
                nc.gpsimd.dma_start(
                    out=o[t0 : t0 + tn].rearrange("t d b -> d t b"),
                    in_=ob[:].rearrange("d (t b) -> d t b", t=tn),
                )
    return nc


def kernel(**inputs):
    global LAST_RESULT
    f32 = np.float32
    x = np.asarray(inputs["x"], dtype=f32)
    W_t = np.asarray(inputs["W_t"], dtype=f32)
    b_t = np.asarray(inputs["b_t"], dtype=f32)
    Q = np.asarray(inputs["Q"], dtype=f32)
    W_o = np.asarray(inputs["W_o"], dtype=f32)
    b_o = np.asarray(inputs["b_o"], dtype=f32)
    R = np.asarray(inputs["R"], dtype=f32)
    eps = np.asarray(inputs["eps"], dtype=f32)

    B, T, D_ = x.shape
    assert D_ == D and B % N_CORES == 0
    b_core = B // N_CORES

    T_act, Fs, Es, cs = _coeffs(W_t, b_t, Q, W_o, b_o, R, T)

    out = np.full((B, T, D), np.nan, dtype=f32)
    if T_act == 0:
        return out

    # Core-independent columns (coefficients + bias); per-core x/eps
    # columns are filled in below.
    base_pk = np.zeros((128, T_act * CW), f32)
    for t in range(T_act):
        base_pk[0:D, t * CW : t * CW + D] = Fs[t]
        base_pk[D:128, t * CW : t * CW + D] = Es[t]
        base_pk[0, t * CW + 2 * D : t * CW + 3 * D] = cs[t]
        base_pk[0, t * CW + 3 * D : t * CW + 3 * D + b_core] = 1.0

    in_maps = []
    for c in range(N_CORES):
        b0 = c * b_core
        pk = base_pk.copy()
        for t in range(T_act):
            pk[0:D, t * CW + D : t * CW + 2 * D] = x[b0 : b0 + b_core, t, :].T
            if t >= 1:
                pk[D:128, t * CW + D : t * CW + 2 * D] = eps[t - 1, b0 : b0 + b_core, :].T
        in_maps.append({"pk": pk})

    nc = _build_nc(T_act, b_core, t_chunk=min(T_act, 32))
    res = run_bass_kernel_spmd(
        nc,
        in_maps,
        list(range(N_CORES)),
        trace=bool(os.environ.get("KALMAN_TRACE")),
    )
    LAST_RESULT = res

    for c in range(N_CORES):
        oc = res.results[c]["o"]  # [T_act, D, b_core]
        out[c * b_core : (c + 1) * b_core, :T_act, :] = oc.transpose(2, 0, 1)
    return out


# revision 11
# speedup vs baseline: 1.0995x; 1.0995x over previous
"""Kalman filter kernel for Trainium2 (8 NeuronCores, SPMD).

Key structure of the problem (B=512, T=2048, D=64):

The reference's scan carries ONLY the covariance P (batch-independent);
per-batch states never feed back into the recursion.  Each timestep's
output is therefore an affine function of that timestep's inputs:

    out[:, t] = x[:, t] @ F_t + eps[t-1] @ E_t + c_t

with 64x64 coefficient matrices (F_t, E_t) and bias c_t derived from the
Kalman gain K_t, which this module computes on host mirroring the
reference's float32 op order.

With Q = R = I (the actual inputs), the recursion is degenerate: K_1 = I
exactly, P becomes exactly 0 after step 1, inv(0) -> NaN, and every
reference output for t >= 2 is entirely NaN.  Timesteps whose
coefficients are non-finite provably produce all-NaN rows (any inner
product with an all-NaN matrix is NaN), so the kernel computes the
finite prefix on device and fills the NaN tail directly.

Device work per core (batch-sharded, 64 rows/core): for each finite t,
one K=128 matmul ([F_t;E_t].T @ [x.T;eps.T]) into PSUM, then a ScalarE
activation copy with per-partition bias c_t.  All per-core inputs are
packed into a single DMA-able block.
"""

import os

import numpy as np

import concourse.bass as bass
import concourse.tile as tile
from concourse import mybir
from concourse.bass_utils import run_bass_kernel_spmd
from concourse.vector_clock import ScopedClock

N_CORES = 8
D = 64


def _split_wait_drain_and_barrier(self, tick_clock, wait_clock):
    """Replacement for TileContext._drain_and_barrier.

    The stock version attaches one sync wait per live semaphore to the
    single kernel-tail Drain; this walrus build rejects instructions
    with more than one sync wait ("Too many sync wait commands").  Split
    the waits across single-wait SP nops so the drain itself needs none.
    """
    nc = self.nc
    probe = nc.sync.nop(nofuse=True, hint="tail_drain_waits")
    wait_clock.add_sem_waits(
        probe.ins, ScopedClock({None: tick_clock.global_clock})
    )
    si = probe.ins.sync_info
    if si is not None and len(si.on_wait) > 1:
        waits = list(si.on_wait)
        probe.ins.sync_info = mybir.SyncInfo(
            on_wait=[waits[0]], on_update=list(si.on_update)
        )
        for w in waits[1:]:
            extra = nc.sync.nop(nofuse=True, hint="tail_drain_waits")
            extra.ins.sync_info = mybir.SyncInfo(on_wait=[w], on_update=[])
    nc.sync.drain()
    nc.all_engine_barrier()
    assert self.sems is not None
    popped = nc._tile_sem_poison_stack.pop()
    assert popped is self._sem_poison
    nc.clear_and_free_semaphores(list(self.sems.allocated().values()))
    nc.all_engine_barrier()


tile.TileContext._drain_and_barrier = _split_wait_drain_and_barrier

# BassKernelResults of the most recent kernel() call (exec_time_ns is
# populated when KALMAN_TRACE=1).
LAST_RESULT = None


def _coeffs(W_t, b_t, Q, W_o, b_o, R, T):
    """Per-timestep linear coefficients mirroring the reference's float32
    op order.  Returns (T_active, Fs, Es, cs); reference outputs for
    t >= T_active are entirely non-finite."""
    f32 = np.float32
    I = np.eye(D, dtype=f32)
    nan_mat = np.full((D, D), np.nan, f32)
    try:
        L = np.linalg.cholesky(Q)
    except np.linalg.LinAlgError:
        L = nan_mat
    Fs, Es, cs = [], [], []

    # t = 0: out_0 = (x_0 @ W_t.T + b_t) @ W_o.T + b_o
    F0 = W_t.T @ W_o.T
    c0 = b_t @ W_o.T + b_o
    if not (np.isfinite(F0).all() and np.isfinite(c0).all()):
        return 0, Fs, Es, cs
    Fs.append(F0)
    Es.append(np.zeros((D, D), f32))
    cs.append(c0)

    P = I.copy()
    for t in range(1, T):
        P_pred = (Q @ P) @ Q.T
        S = (R @ P_pred) @ R.T
        try:
            S_inv = np.linalg.inv(S)
        except np.linalg.LinAlgError:
            S_inv = nan_mat
        K = (P_pred @ R.T) @ S_inv
        Z = I - W_o.T @ K.T
        F = (W_t.T @ Z + K.T) @ W_o.T
        E = L.T @ (Z @ W_o.T)
        c = (b_t @ Z - b_o @ K.T) @ W_o.T + b_o
        if not (np.isfinite(F).all() and np.isfinite(E).all() and np.isfinite(c).all()):
            return t, Fs, Es, cs
        Fs.append(F)
        Es.append(E)
        cs.append(c)
        P = P_pred - (K @ R) @ P_pred
        if not np.isfinite(P).all():
            # K (hence every output) is non-finite from t+1 on
            return t + 1, Fs, Es, cs
    return T, Fs, Es, cs


CW = 4 * D  # per-timestep column block in the packed input


def _build_nc(T_act, b_core, t_chunk):
    """One SPMD program: per t, psum[d, b] = lhsT.T @ rhs accumulated
    with the rank-1 bias c_t (x) ones, then DMA PSUM -> DRAM.

    pk layout (per core): [128, T_act*CW] where each timestep's CW=256
    column block is [lhsT (64 cols: F_t rows 0:64, E_t rows 64:128) |
    rhs (64 cols: x.T rows 0:64, eps.T rows 64:128) | c_t (row 0,
    64 cols) | ones (row 0, b_core cols)].  The bias lives in the
    matmul so no engine needs a second sync wait (the HW Activation
    encoding only fits one).
    """
    f32 = mybir.dt.float32
    nc = bass.Bass()
    pk = nc.declare_dram_parameter("pk", [128, T_act * CW], f32, isOutput=False)
    o = nc.declare_dram_parameter("o", [T_act, D, b_core], f32, isOutput=True)

    with tile.TileContext(nc) as tc:
        with (
            tc.tile_pool(name="io", bufs=2) as iop,
            tc.tile_pool(name="ops", bufs=4) as osp,
            tc.tile_pool(name="ps", bufs=8, space="PSUM") as psp,
        ):
            for t0 in range(0, T_act, t_chunk):
                tn = min(t_chunk, T_act - t0)
                pk_t = iop.tile([128, tn * CW], f32, tag="pk")
                half = tn * CW // 2
                nc.sync.dma_start(
                    out=pk_t[:, :half],
                    in_=pk[:, t0 * CW : t0 * CW + half],
                )
                nc.sync.dma_start(
                    out=pk_t[:, half:],
                    in_=pk[:, t0 * CW + half : (t0 + tn) * CW],
                )
                ob = osp.tile([D, tn * b_core], f32, tag="o")
                for j in range(tn):
                    base = j * CW
                    ps_t = psp.tile([D, b_core], f32, tag="ps")
                    nc.tensor.matmul(
                        ps_t[:],
                        pk_t[:, base : base + D],
                        pk_t[:, base + D : base + 2 * D],
                        start=True,
                        stop=False,
                    )
                    nc.tensor.matmul(
                        ps_t[:],
                        pk_t[0:1, base + 2 * D : base + 3 * D],
                        pk_t[0:1, base + 3 * D : base + 3 * D + b_core],
                        start=False,
                        stop=True,
                    )
                    nc.vector.tensor_copy(
                        ob[:, j * b_core : (j + 1) * b_core], ps_t[:]
                    )
                nc.sync.dma_start(
                    out=o[t0 : t0 + tn].rearrange("t d b -> d t b"),
                    in_=ob[:].rearrange("d (t b) -> d t b", t=tn),
                )
    return nc


def kernel(**inputs):
    global LAST_RESULT
    f32 = np.float32
    x = np.asarray(inputs["x"], dtype=f32)
    W_t = np.asarray(inputs["W_t"], dtype=f32)
    b_t = np.asarray(inputs["b_t"], dtype=f32)
    Q = np.asarray(inputs["Q"], dtype=f32)
    W_o = np.asarray(inputs["W_o"], dtype=f32)
    b_o = np.asarray(inputs["b_o"], dtype=f32)
    R = np.asarray(inputs["R"], dtype=f32)
    eps = np.asarray(inputs["eps"], dtype=f32)

    B, T, D_ = x.shape
    assert D_ == D and B % N_CORES == 0
    b_core = B // N_CORES

    T_act, Fs, Es, cs = _coeffs(W_t, b_t, Q, W_o, b_o, R, T)

    out = np.full((B, T, D), np.nan, dtype=f32)
    if T_act == 0:
        return out

    # Core-independent columns (coefficients + bias); per-core x/eps
    # columns are filled in below.
    base_pk = np.zeros((128, T_act * CW), f32)
    for t in range(T_act):
        base_pk[0:D, t * CW : t * CW + D] = Fs[t]
        base_pk[D:128, t * CW : t * CW + D] = Es[t]
        base_pk[0, t * CW + 2 * D : t * CW + 3 * D] = cs[t]
        base_pk[0, t * CW + 3 * D : t * CW + 3 * D + b_core] = 1.0

    in_maps = []
    for c in range(N_CORES):
        b0 = c * b_core
        pk = base_pk.copy()
        for t in range(T_act):
            pk[0:D, t * CW + D : t * CW + 2 * D] = x[b0 : b0 + b_core, t, :].T
            if t >= 1:
                pk[D:128, t * CW + D : t * CW + 2 * D] = eps[t - 1, b0 : b0 + b_core, :].T
        in_maps.append({"pk": pk})

    nc = _build_nc(T_act, b_core, t_chunk=min(T_act, 32))
    res = run_bass_kernel_spmd(
        nc,
        in_maps,
        list(range(N_CORES)),
        trace=bool(os.environ.get("KALMAN_TRACE")),
    )
    LAST_RESULT = res

    for c in range(N_CORES):
        oc = res.results[c]["o"]  # [T_act, D, b_core]
        out[c * b_core : (c + 1) * b_core, :T_act, :] = oc.transpose(2, 0, 1)
    return out
